# revision 1
# baseline (speedup 1.0000x reference)
"""Trainium2 Bass kernel for nn_AnaphoricityScorer (masked attention logits).

Computes attn = mask(q @ k.T) where
    q = (X @ Wq.T + bq) / sqrt(E),  k = X @ Wk.T + bk,  X = all_mentions
and mask sets entries strictly above the diagonal to -inf.

Strategy (8 NeuronCores, SPMD — one NEFF, per-core data):
  * Algebra: q @ k.T = X @ W2 @ X.T + s[r] + v2[n], with
        W2 = Wq.T @ Wk / sqrt(E)   (fused on host, tiny)
        s  = q @ bk, v2 = (bq @ Wk @ X.T)/sqrt(E)  (rank-1 bias terms; zero
        for this problem's inputs, added on host when nonzero).
    The device therefore runs ONE projection t = X_loc @ W2 and one big
    t @ X.T instead of two projections + attention.
  * Causal triangle: ~44% of the N x N output is -inf; those tiles are never
    computed.  Row-blocks of 128 are dealt interleaved (core c gets blocks
    c, c+8, ..., c+56) and each core's j-th block computes columns
    0..1024*(j+1) — identical shapes on every core, so one NEFF serves all 8.
  * Compute in bf16 (TensorE 4x faster than fp32), accumulate fp32 in PSUM.
"""

import sys

if "/opt/trn_rl_repo" not in sys.path:
    sys.path.insert(0, "/opt/trn_rl_repo")

import ml_dtypes
import numpy as np

BF16 = ml_dtypes.bfloat16
N, E, P, NCORES = 8192, 1024, 128, 8
NBLK = N // (P * NCORES)  # 8 row-blocks of 128 rows per core
NEG_INF = np.float32(-np.inf)

_CACHE: dict = {}


def _build_nc():
    import concourse.mybir as mybir
    import concourse.tile as tile
    from concourse import bacc

    f32 = mybir.dt.float32
    bf = mybir.dt.bfloat16

    nc = bacc.Bacc("TRN2", target_bir_lowering=False, debug=False,
                   num_devices=NCORES)
    xt = nc.dram_tensor("xt", [8, E, 1024], bf, kind="ExternalInput")
    xtloc = nc.dram_tensor("xtloc", [E, 1024], bf, kind="ExternalInput")
    w2 = nc.dram_tensor("w2", [E, 1024], bf, kind="ExternalInput")
    mask = nc.dram_tensor("mask", [P, 1024], f32, kind="ExternalInput")
    out = nc.dram_tensor("out", [1024, N], f32, kind="ExternalOutput")
    outap = out.ap()

    with tile.TileContext(nc) as tc:
        with (
            tc.tile_pool(name="const", bufs=1) as constp,
            tc.tile_pool(name="xtp", bufs=3) as xtp,
            tc.tile_pool(name="obp", bufs=4) as obp,
            tc.tile_pool(name="psp", bufs=6, space="PSUM") as psp,
        ):
            w2_sb = constp.tile([P, 8, 1024], bf)
            nc.sync.dma_start(w2_sb[:], w2.ap().rearrange("(eo p) n -> p eo n", p=P))
            xl_sb = constp.tile([P, 8, 1024], bf)
            nc.sync.dma_start(xl_sb[:], xtloc.ap().rearrange("(eo p) n -> p eo n", p=P))
            mask_sb = constp.tile([P, 1024], f32)
            nc.sync.dma_start(mask_sb[:], mask.ap())
            tt_sb = constp.tile([P, 8, 1024], bf)

            # Phase 1: tT[i, r] = (X_loc @ W2).T, kept transposed for phase 2.
            for ic in range(8):
                for rh in range(2):
                    ps = psp.tile([P, 512], f32)
                    for e in range(8):
                        nc.tensor.matmul(
                            ps[:],
                            w2_sb[:, e, ic * P:(ic + 1) * P],
                            xl_sb[:, e, rh * 512:(rh + 1) * 512],
                            start=(e == 0),
                            stop=(e == 7),
                        )
                    nc.vector.tensor_copy(tt_sb[:, ic, rh * 512:(rh + 1) * 512], ps[:])

            # Phase 2: attn row-block j (cols chunk cc) = t_j @ X_cc.T
            for cc in range(8):
                xt_sb = xtp.tile([P, 8, 1024], bf)
                nc.sync.dma_start(
                    xt_sb[:], xt.ap()[cc].rearrange("(eo p) n -> p eo n", p=P)
                )
                for j in range(cc, 8):
                    ob = obp.tile([P, 1024], f32)
                    for nh in range(2):
                        ps = psp.tile([P, 512], f32)
                        for ic in range(8):
                            nc.tensor.matmul(
                                ps[:],
                                tt_sb[:, ic, j * P:(j + 1) * P],
                                xt_sb[:, ic, nh * 512:(nh + 1) * 512],
                                start=(ic == 0),
                                stop=(ic == 7),
                            )
                        dst = ob[:, nh * 512:(nh + 1) * 512]
                        if cc == j:
                            # diagonal chunk: fuse the causal mask add
                            nc.vector.tensor_add(
                                dst, ps[:], mask_sb[:, nh * 512:(nh + 1) * 512]
                            )
                        else:
                            nc.vector.tensor_copy(dst, ps[:])
                    nc.sync.dma_start(
                        outap[j * P:(j + 1) * P, cc * 1024:(cc + 1) * 1024], ob[:]
                    )
    nc.compile()
    return nc


def _get_nc():
    if "nc" not in _CACHE:
        _CACHE["nc"] = _build_nc()
    return _CACHE["nc"]


def kernel(all_mentions, Wq, bq, Wk, bk):
    from concourse.bass_utils import run_bass_kernel_spmd

    X = np.asarray(all_mentions, dtype=np.float32)
    Wq = np.asarray(Wq, dtype=np.float32)
    Wk = np.asarray(Wk, dtype=np.float32)
    bq = np.asarray(bq, dtype=np.float32)
    bk = np.asarray(bk, dtype=np.float32)

    scale = np.float32(1.0 / np.sqrt(E))
    W2 = ((Wq.T @ Wk) * scale).astype(BF16)  # [E, E], fused q/k projection

    Xb = X.astype(BF16)
    XbT = np.ascontiguousarray(Xb.T)  # [E, N]
    # xt[cc, e, m] = X[cc*1024 + m, e]
    xt_np = np.ascontiguousarray(XbT.reshape(E, 8, 1024).swapaxes(0, 1))

    in_maps = []
    for c in range(NCORES):
        rows = np.concatenate(
            [np.arange((c + 8 * j) * P, (c + 8 * j + 1) * P) for j in range(NBLK)]
        )
        xtloc_np = np.ascontiguousarray(Xb[rows, :].T)  # [E, 1024]
        # mask[i, m] = -inf where (global col) > (global row)  <=>  m > c*128 + i
        i_idx = np.arange(P)[:, None]
        m_idx = np.arange(1024)[None, :]
        mask_np = np.where(m_idx > c * P + i_idx, NEG_INF, np.float32(0.0))
        mask_np = mask_np.astype(np.float32)
        in_maps.append(
            {"xt": xt_np, "xtloc": xtloc_np, "w2": W2, "mask": mask_np}
        )

    nc = _get_nc()
    res = run_bass_kernel_spmd(nc, in_maps, core_ids=list(range(NCORES)))

    # Assemble: device wrote rows of block b=c+8j at cols [0, 1024*(j+1));
    # everything else in those rows is -inf by the causal mask.
    out_full = np.full((N, N), NEG_INF, dtype=np.float32)
    for c in range(NCORES):
        dev = res.results[c]["out"]
        for j in range(NBLK):
            b = c + 8 * j
            w = 1024 * (j + 1)
            out_full[b * P:(b + 1) * P, :w] = dev[j * P:(j + 1) * P, :w]

    # Rank-1 bias terms (zero for this problem's inputs; exact in general).
    if np.any(bq) or np.any(bk):
        v2 = (X @ (bq @ Wk)) * scale  # [N] per-column term
        s = ((X @ Wq.T + bq) * scale) @ bk  # [N] per-row term
        out_full += s[:, None]
        out_full += v2[None, :]

    return out_full


# revision 8
# speedup vs baseline: 1.0554x; 1.0554x over previous
"""Trainium2 Bass kernel for nn_AnaphoricityScorer (masked attention logits).

Computes attn = causal_mask(q @ k.T) where
    q = (X @ Wq.T + bq) / sqrt(E),  k = X @ Wk.T + bk,  X = all_mentions
and the mask sets entries strictly above the diagonal to -inf.

Strategy (8 NeuronCores, 2 NEFF variants, per-core data):
  * Algebra: q @ k.T = X @ W2 @ X.T + s[r] + v2[n], with
        W2 = Wq.T @ Wk / sqrt(E)          (fused on host, tiny)
        s = q @ bk, v2 = (bq @ Wk @ X.T)/sqrt(E)  (rank-1 bias terms; zero for
        this problem's inputs, added on host when nonzero).
    The device runs ONE projection t = X_loc @ W2 plus one big t @ X.T.
  * Causal triangle: ~44% of the N x N output is -inf and is never computed.
    Row-blocks of 128 rows are assigned to cores so that each core's 8 blocks
    have column needs (in 512-col units) matching a fixed per-variant multiset:
        variant A (cores 0-3): {16,15,14,13,4,3,2,1}
        variant B (cores 4-7): {12,11,10,9,8,7,6,5}
    Both sum to 68 [128x512] output tiles -> identical wall time, and
    sum_b ceil((b+1)/4) = 544 matmuls/core is the uniform-graph minimum at
    512-column granularity.  The inner loop streams 512-column chunks of X.T
    in natural order; position p consumes chunks 0..S[p]-1.
  * Masking and -inf fill happen on the host during unsharding (the device
    computes only finite logits); compute is bf16 with fp32 PSUM accumulation.
  * A few dummy matmuls at kernel start warm the PE HAM clock-gate to 2.4GHz
    while the first input DMAs are still in flight.
"""

import sys

if "/opt/trn_rl_repo" not in sys.path:
    sys.path.insert(0, "/opt/trn_rl_repo")

import ml_dtypes
import numpy as np

BF16 = ml_dtypes.bfloat16
N, E, P, NCORES = 8192, 1024, 128, 8
NBLK = 8  # row-blocks of 128 rows per core
NEG_INF = np.float32(-np.inf)

# variant -> per-position column need in 512-col units (descending)
SLOTS = {
    "A": [16, 15, 14, 13, 4, 3, 2, 1],
    "B": [12, 11, 10, 9, 8, 7, 6, 5],
}


def blocks_for_core(c):
    """Row-blocks (of 128 rows) owned by core c, in position order."""
    var = "A" if c < 4 else "B"
    cc = c if c < 4 else c - 4
    return [4 * (k - 1) + cc for k in SLOTS[var]], var


_CACHE: dict = {}


def _build_nc(var):
    import concourse.mybir as mybir
    import concourse.tile as tile
    from concourse import bacc

    f32 = mybir.dt.float32
    bf = mybir.dt.bfloat16
    slots = SLOTS[var]
    nchunks = max(slots)  # 16 for A, 12 for B

    nc = bacc.Bacc("TRN2", target_bir_lowering=False, debug=False,
                   num_devices=4)
    xt = nc.dram_tensor("xt", [nchunks, E, 512], bf, kind="ExternalInput")
    xtloc = nc.dram_tensor("xtloc", [E, 1024], bf, kind="ExternalInput")
    w2 = nc.dram_tensor("w2", [E, 1024], bf, kind="ExternalInput")
    out = nc.dram_tensor("out", [1024, 512 * nchunks], f32,
                         kind="ExternalOutput")
    outap = out.ap()

    with tile.TileContext(nc) as tc:
        with (
            tc.tile_pool(name="const", bufs=1) as constp,
            tc.tile_pool(name="xtp", bufs=4) as xtp,
            tc.tile_pool(name="obp", bufs=6) as obp,
            tc.tile_pool(name="psp", bufs=8, space="PSUM") as psp,
        ):
            # PE warm-up: dummy matmuls on uninitialized SBUF flip the HAM
            # clock gate to 8/8 while the first input DMAs land.  Results go
            # to a scratch PSUM tile that is never read.
            wsrc = constp.tile([P, 512], bf)
            nc.gpsimd.memset(wsrc[:], 0.0)
            wps = psp.tile([P, 512], f32, tag="ps")
            for _ in range(10):
                nc.tensor.matmul(wps[:], wsrc[:, :P], wsrc[:], start=True,
                                 stop=True)

            w2_sb = constp.tile([P, 8, 1024], bf)
            xl_sb = constp.tile([P, 8, 1024], bf)
            w2_ap = w2.ap().rearrange("(eo p) n -> p eo n", p=P)
            xl_ap = xtloc.ap().rearrange("(eo p) n -> p eo n", p=P)
            # per-e-chunk DMAs (scalar queue) so the first matmul can start
            # after ~512KB instead of 4MB
            for e in range(8):
                nc.scalar.dma_start(w2_sb[:, e:e + 1, :], w2_ap[:, e:e + 1, :])
                nc.scalar.dma_start(xl_sb[:, e:e + 1, :], xl_ap[:, e:e + 1, :])
            tt_sb = constp.tile([P, 8, 1024], bf)

            # Phase 1: tT = (X_loc @ W2).T, e-outer over 8 PSUM banks so the
            # PE starts on chunk 0 while later chunks are still in flight.
            for rh in range(2):
                pss = [psp.tile([P, 512], f32, name=f"ttps_{rh}_{k}", tag="ps")
                       for k in range(8)]
                for e in range(8):
                    for ic in range(8):
                        nc.tensor.matmul(
                            pss[ic][:],
                            w2_sb[:, e, ic * P:(ic + 1) * P],
                            xl_sb[:, e, rh * 512:(rh + 1) * 512],
                            start=(e == 0),
                            stop=(e == 7),
                        )
                for ic in range(8):
                    dst = tt_sb[:, ic, rh * 512:(rh + 1) * 512]
                    if ic % 2 == 0:
                        nc.vector.tensor_copy(dst, pss[ic][:])
                    else:
                        nc.scalar.copy(dst, pss[ic][:])

            # Phase 2: stream 512-col chunks of X.T; chunk t feeds every
            # position p with slots[p] > t.
            for t in range(nchunks):
                xt_sb = xtp.tile([P, 8, 512], bf)
                nc.sync.dma_start(
                    xt_sb[:], xt.ap()[t].rearrange("(eo p) n -> p eo n", p=P)
                )
                for pos in range(8):
                    if slots[pos] <= t:
                        continue
                    ps = psp.tile([P, 512], f32, tag="ps")
                    for ic in range(8):
                        nc.tensor.matmul(
                            ps[:],
                            tt_sb[:, ic, pos * P:(pos + 1) * P],
                            xt_sb[:, ic, :],
                            start=(ic == 0),
                            stop=(ic == 7),
                        )
                    ob = obp.tile([P, 512], f32)
                    if pos % 2 == 0:
                        nc.vector.tensor_copy(ob[:], ps[:])
                    else:
                        nc.scalar.copy(ob[:], ps[:])
                    nc.sync.dma_start(
                        outap[pos * P:(pos + 1) * P, t * 512:(t + 1) * 512],
                        ob[:],
                    )
    nc.compile()
    return nc


def _make_exec(nc, devices):
    """Build a cached jitted executable for one variant (4 devices)."""
    import jax
    from jax.sharding import Mesh, PartitionSpec
    try:
        from jax.experimental.shard_map import shard_map
    except ImportError:
        from jax.shard_map import shard_map
    import concourse.mybir as mybir
    from concourse import bass2jax

    bass2jax.install_neuronx_cc_hook()

    partition_name = (nc.partition_id_tensor.name
                      if nc.partition_id_tensor else None)
    in_names, out_names, out_avals, zero_shapes = [], [], [], []
    for alloc in nc.m.functions[0].allocations:
        if not isinstance(alloc, mybir.MemoryLocationSet):
            continue
        name = alloc.memorylocations[0].name
        if alloc.kind == "ExternalInput":
            if name != partition_name:
                in_names.append(name)
        elif alloc.kind == "ExternalOutput":
            out_names.append(name)
            shape = tuple(alloc.tensor_shape)
            dtype = mybir.dt.np(alloc.dtype)
            out_avals.append(jax.core.ShapedArray(shape, dtype))
            zero_shapes.append((shape, dtype))
    n_params = len(in_names)
    all_names = in_names + out_names
    if partition_name is not None:
        all_names = all_names + [partition_name]
    donate = tuple(range(n_params, n_params + len(out_names)))

    def _body(*args):
        operands = list(args)
        if partition_name is not None:
            operands.append(bass2jax.partition_id_tensor())
        outs = bass2jax._bass_exec_p.bind(
            *operands,
            out_avals=tuple(out_avals),
            in_names=tuple(all_names),
            out_names=tuple(out_names),
            lowering_input_output_aliases=(),
            sim_require_finite=True,
            sim_require_nnan=True,
            nc=nc,
        )
        return tuple(outs)

    n_cores = len(devices)
    mesh = Mesh(np.asarray(devices), ("core",))
    specs = (PartitionSpec("core"),) * (n_params + len(out_names))
    jitted = jax.jit(
        shard_map(_body, mesh=mesh, in_specs=specs,
                  out_specs=(PartitionSpec("core"),) * len(out_names),
                  check_rep=False),
        donate_argnums=donate, keep_unused=True,
    )

    def run(in_maps):
        concat_in = [
            np.concatenate([np.asarray(m[k]) for m in in_maps], axis=0)
            for k in in_names
        ]
        zeros = [np.zeros((n_cores * s[0], *s[1:]), dt)
                 for (s, dt) in zero_shapes]
        out_arrs = jitted(*concat_in, *zeros)
        return out_arrs

    return run


def _get_runners():
    if "runners" not in _CACHE:
        import jax

        devs = jax.devices()
        nc_a = _build_nc("A")
        nc_b = _build_nc("B")
        _CACHE["runners"] = (
            _make_exec(nc_a, devs[0:4]),
            _make_exec(nc_b, devs[4:8]),
        )
    return _CACHE["runners"]


_TRI = np.triu(np.ones((P, P), dtype=bool), k=1)


def kernel(all_mentions, Wq, bq, Wk, bk):
    X = np.asarray(all_mentions, dtype=np.float32)
    Wq = np.asarray(Wq, dtype=np.float32)
    Wk = np.asarray(Wk, dtype=np.float32)
    bq = np.asarray(bq, dtype=np.float32)
    bk = np.asarray(bk, dtype=np.float32)

    scale = np.float32(1.0 / np.sqrt(E))
    W2 = ((Wq.T @ Wk) * scale).astype(BF16)  # [E, E] fused projection

    Xb = X.astype(BF16)
    XbT = np.ascontiguousarray(Xb.T)  # [E, N]
    # xt[t, e, m] = X[512 t + m, e]; variant B only consumes chunks 0..11
    xt_np = np.ascontiguousarray(XbT.reshape(E, 16, 512).swapaxes(0, 1))
    xt_np_b = np.ascontiguousarray(xt_np[:12])

    in_maps_a, in_maps_b = [], []
    for c in range(NCORES):
        blocks, var = blocks_for_core(c)
        rows = np.concatenate(
            [np.arange(b * P, (b + 1) * P) for b in blocks])
        xtloc_np = np.ascontiguousarray(Xb[rows, :].T)  # [E, 1024]
        m = {
            "xt": xt_np if var == "A" else xt_np_b,
            "xtloc": xtloc_np,
            "w2": W2,
        }
        (in_maps_a if var == "A" else in_maps_b).append(m)

    run_a, run_b = _get_runners()
    # dispatch both variants; execution overlaps on disjoint device sets
    arrs_a = run_a(in_maps_a)
    arrs_b = run_b(in_maps_b)
    out_a = np.asarray(arrs_a[0]).reshape(4, 1024, 512 * 16)
    out_b = np.asarray(arrs_b[0]).reshape(4, 1024, 512 * 12)

    # Unshard: copy each block's valid columns, apply the triangular mask on
    # the diagonal 128x128 sub-block, fill the rest with -inf.
    out_full = np.full((N, N), NEG_INF, dtype=np.float32)
    for c in range(NCORES):
        blocks, var = blocks_for_core(c)
        dev = out_a[c] if var == "A" else out_b[c - 4]
        for pos, b in enumerate(blocks):
            w = (b + 1) * P
            blk = out_full[b * P:(b + 1) * P]
            blk[:, :w] = dev[pos * P:(pos + 1) * P, :w]
            blk[:, b * P:(b + 1) * P][_TRI] = NEG_INF

    # Rank-1 bias terms (zero for this problem's inputs; exact in general).
    if np.any(bq) or np.any(bk):
        v2 = (X @ (bq @ Wk)) * scale  # per-column term
        s = ((X @ Wq.T + bq) * scale) @ bk  # per-row term
        out_full += s[:, None]
        out_full += v2[None, :]

    return out_full


# revision 10
# speedup vs baseline: 1.2298x; 1.1653x over previous
"""Trainium2 Bass kernel for nn_AnaphoricityScorer (masked attention logits).

Computes attn = causal_mask(q @ k.T) where
    q = (X @ Wq.T + bq) / sqrt(E),  k = X @ Wk.T + bk,  X = all_mentions
and the mask sets entries strictly above the diagonal to -inf.

Strategy (8 NeuronCores, 2 NEFF variants, per-core data):
  * Algebra: q @ k.T = X @ W2 @ X.T + s[r] + v2[n], with
        W2 = Wq.T @ Wk / sqrt(E)          (fused on host, tiny)
        s = q @ bk, v2 = (bq @ Wk @ X.T)/sqrt(E)  (rank-1 bias terms; zero for
        this problem's inputs, added on host when nonzero).
    The device runs ONE projection t = X_loc @ W2 plus one big t @ X.T.
  * Causal triangle: ~44% of the N x N output is -inf and is never computed.
    Row-blocks of 128 rows are assigned to cores so that each core's 8 blocks
    have column needs (in 512-col units) matching a fixed per-variant multiset:
        variant A (cores 0-3): {16,15,14,13,4,3,2,1}
        variant B (cores 4-7): {12,11,10,9,8,7,6,5}
    Both sum to 68 [128x512] output tiles -> identical wall time, and
    sum_b ceil((b+1)/4) = 544 matmuls/core is the uniform-graph minimum at
    512-column granularity.  The inner loop streams 512-column chunks of X.T
    in natural order; position p consumes chunks 0..S[p]-1.
  * Masking and -inf fill happen on the host during unsharding (the device
    computes only finite logits); compute is bf16 with fp32 PSUM accumulation.
  * A few dummy matmuls at kernel start warm the PE HAM clock-gate to 2.4GHz
    while the first input DMAs are still in flight.
"""

import sys

if "/opt/trn_rl_repo" not in sys.path:
    sys.path.insert(0, "/opt/trn_rl_repo")

import ml_dtypes
import numpy as np

BF16 = ml_dtypes.bfloat16
N, E, P, NCORES = 8192, 1024, 128, 8
NBLK = 8  # row-blocks of 128 rows per core
NEG_INF = np.float32(-np.inf)

# variant -> per-position column need in 512-col units (descending)
SLOTS = {
    "A": [16, 15, 14, 13, 4, 3, 2, 1],
    "B": [12, 11, 10, 9, 8, 7, 6, 5],
}


def blocks_for_core(c):
    """Row-blocks (of 128 rows) owned by core c, in position order."""
    var = "A" if c < 4 else "B"
    cc = c if c < 4 else c - 4
    return [4 * (k - 1) + cc for k in SLOTS[var]], var


_CACHE: dict = {}


def _build_nc(var):
    import concourse.mybir as mybir
    import concourse.tile as tile
    from concourse import bacc

    f32 = mybir.dt.float32
    bf = mybir.dt.bfloat16
    slots = SLOTS[var]
    nchunks = max(slots)  # 16 for A, 12 for B

    nc = bacc.Bacc("TRN2", target_bir_lowering=False, debug=False,
                   num_devices=4)
    xt = nc.dram_tensor("xt", [nchunks, E, 512], bf, kind="ExternalInput")
    xtloc = nc.dram_tensor("xtloc", [E, 1024], bf, kind="ExternalInput")
    w2 = nc.dram_tensor("w2", [E, 1024], bf, kind="ExternalInput")
    out = nc.dram_tensor("out", [1024, 512 * nchunks], f32,
                         kind="ExternalOutput")
    outap = out.ap()

    with tile.TileContext(nc) as tc:
        with (
            tc.tile_pool(name="const", bufs=1) as constp,
            tc.tile_pool(name="xtp", bufs=4) as xtp,
            tc.tile_pool(name="obp", bufs=6) as obp,
            tc.tile_pool(name="psp", bufs=8, space="PSUM") as psp,
        ):
            # PE warm-up: dummy matmuls on uninitialized SBUF flip the HAM
            # clock gate to 8/8 while the first input DMAs land.  Results go
            # to a scratch PSUM tile that is never read.
            wsrc = constp.tile([P, 512], bf)
            nc.gpsimd.memset(wsrc[:], 0.0)
            wps0 = psp.tile([P, 512], f32, tag="ps")
            wps1 = psp.tile([P, 512], f32, tag="ps")
            for k in range(4):
                # alternate banks so fill overlaps drain (~216ns spacing)
                nc.tensor.matmul(wps0[:], wsrc[:, :P], wsrc[:],
                                 start=(k == 0), stop=(k == 3))
                nc.tensor.matmul(wps1[:], wsrc[:, :P], wsrc[:],
                                 start=(k == 0), stop=(k == 3))

            w2_sb = constp.tile([P, 8, 1024], bf)
            xl_sb = constp.tile([P, 8, 1024], bf)
            w2_ap = w2.ap().rearrange("(eo p) n -> p eo n", p=P)
            xl_ap = xtloc.ap().rearrange("(eo p) n -> p eo n", p=P)
            # per-e-chunk DMAs (scalar queue) so the first matmul can start
            # after ~512KB instead of 4MB
            for e in range(8):
                nc.scalar.dma_start(w2_sb[:, e:e + 1, :], w2_ap[:, e:e + 1, :])
                nc.scalar.dma_start(xl_sb[:, e:e + 1, :], xl_ap[:, e:e + 1, :])
            tt_sb = constp.tile([P, 8, 1024], bf)

            # Phase 1: tT = (X_loc @ W2).T, e-outer over 8 PSUM banks so the
            # PE starts on chunk 0 while later chunks are still in flight.
            for rh in range(2):
                pss = [psp.tile([P, 512], f32, name=f"ttps_{rh}_{k}", tag="ps")
                       for k in range(8)]
                for e in range(8):
                    for ic in range(8):
                        nc.tensor.matmul(
                            pss[ic][:],
                            w2_sb[:, e, ic * P:(ic + 1) * P],
                            xl_sb[:, e, rh * 512:(rh + 1) * 512],
                            start=(e == 0),
                            stop=(e == 7),
                        )
                for ic in range(8):
                    dst = tt_sb[:, ic, rh * 512:(rh + 1) * 512]
                    if ic % 2 == 0:
                        nc.vector.tensor_copy(dst, pss[ic][:])
                    else:
                        nc.scalar.copy(dst, pss[ic][:])

            # Phase 2: stream 512-col chunks of X.T; chunk t feeds every
            # position p with slots[p] > t.
            for t in range(nchunks):
                xt_sb = xtp.tile([P, 8, 512], bf)
                nc.sync.dma_start(
                    xt_sb[:], xt.ap()[t].rearrange("(eo p) n -> p eo n", p=P)
                )
                active = [pos for pos in range(8) if slots[pos] > t]
                # process positions in pairs so consecutive matmuls hit
                # alternating PSUM banks (same-bank accumulation serializes
                # fill against drain: 259ns/MM instead of 216ns/MM)
                for g in range(0, len(active), 2):
                    grp = active[g:g + 2]
                    pss = [psp.tile([P, 512], f32, name=f"aps_{t}_{g}_{k}",
                                    tag="ps") for k in range(len(grp))]
                    for ic in range(8):
                        for ps, pos in zip(pss, grp):
                            nc.tensor.matmul(
                                ps[:],
                                tt_sb[:, ic, pos * P:(pos + 1) * P],
                                xt_sb[:, ic, :],
                                start=(ic == 0),
                                stop=(ic == 7),
                            )
                    for ps, pos in zip(pss, grp):
                        ob = obp.tile([P, 512], f32)
                        if pos % 2 == 0:
                            nc.vector.tensor_copy(ob[:], ps[:])
                        else:
                            nc.scalar.copy(ob[:], ps[:])
                        nc.sync.dma_start(
                            outap[pos * P:(pos + 1) * P,
                                  t * 512:(t + 1) * 512],
                            ob[:],
                        )
    nc.compile()
    return nc


def _make_exec(nc, devices):
    """Build a cached jitted executable for one variant (4 devices)."""
    import jax
    from jax.sharding import Mesh, PartitionSpec
    try:
        from jax.experimental.shard_map import shard_map
    except ImportError:
        from jax.shard_map import shard_map
    import concourse.mybir as mybir
    from concourse import bass2jax

    bass2jax.install_neuronx_cc_hook()

    partition_name = (nc.partition_id_tensor.name
                      if nc.partition_id_tensor else None)
    in_names, out_names, out_avals, zero_shapes = [], [], [], []
    for alloc in nc.m.functions[0].allocations:
        if not isinstance(alloc, mybir.MemoryLocationSet):
            continue
        name = alloc.memorylocations[0].name
        if alloc.kind == "ExternalInput":
            if name != partition_name:
                in_names.append(name)
        elif alloc.kind == "ExternalOutput":
            out_names.append(name)
            shape = tuple(alloc.tensor_shape)
            dtype = mybir.dt.np(alloc.dtype)
            out_avals.append(jax.core.ShapedArray(shape, dtype))
            zero_shapes.append((shape, dtype))
    n_params = len(in_names)
    all_names = in_names + out_names
    if partition_name is not None:
        all_names = all_names + [partition_name]
    donate = tuple(range(n_params, n_params + len(out_names)))

    def _body(*args):
        operands = list(args)
        if partition_name is not None:
            operands.append(bass2jax.partition_id_tensor())
        outs = bass2jax._bass_exec_p.bind(
            *operands,
            out_avals=tuple(out_avals),
            in_names=tuple(all_names),
            out_names=tuple(out_names),
            lowering_input_output_aliases=(),
            sim_require_finite=True,
            sim_require_nnan=True,
            nc=nc,
        )
        return tuple(outs)

    n_cores = len(devices)
    mesh = Mesh(np.asarray(devices), ("core",))
    specs = (PartitionSpec("core"),) * (n_params + len(out_names))
    jitted = jax.jit(
        shard_map(_body, mesh=mesh, in_specs=specs,
                  out_specs=(PartitionSpec("core"),) * len(out_names),
                  check_rep=False),
        donate_argnums=donate, keep_unused=True,
    )

    def run(in_maps):
        concat_in = [
            np.concatenate([np.asarray(m[k]) for m in in_maps], axis=0)
            for k in in_names
        ]
        zeros = [np.zeros((n_cores * s[0], *s[1:]), dt)
                 for (s, dt) in zero_shapes]
        out_arrs = jitted(*concat_in, *zeros)
        return out_arrs

    return run


def _get_runners():
    if "runners" not in _CACHE:
        import jax

        devs = jax.devices()
        nc_a = _build_nc("A")
        nc_b = _build_nc("B")
        _CACHE["runners"] = (
            _make_exec(nc_a, devs[0:4]),
            _make_exec(nc_b, devs[4:8]),
        )
    return _CACHE["runners"]


_TRI = np.triu(np.ones((P, P), dtype=bool), k=1)


def kernel(all_mentions, Wq, bq, Wk, bk):
    X = np.asarray(all_mentions, dtype=np.float32)
    Wq = np.asarray(Wq, dtype=np.float32)
    Wk = np.asarray(Wk, dtype=np.float32)
    bq = np.asarray(bq, dtype=np.float32)
    bk = np.asarray(bk, dtype=np.float32)

    scale = np.float32(1.0 / np.sqrt(E))
    W2 = ((Wq.T @ Wk) * scale).astype(BF16)  # [E, E] fused projection

    Xb = X.astype(BF16)
    XbT = np.ascontiguousarray(Xb.T)  # [E, N]
    # xt[t, e, m] = X[512 t + m, e]; variant B only consumes chunks 0..11
    xt_np = np.ascontiguousarray(XbT.reshape(E, 16, 512).swapaxes(0, 1))
    xt_np_b = np.ascontiguousarray(xt_np[:12])

    in_maps_a, in_maps_b = [], []
    for c in range(NCORES):
        blocks, var = blocks_for_core(c)
        rows = np.concatenate(
            [np.arange(b * P, (b + 1) * P) for b in blocks])
        xtloc_np = np.ascontiguousarray(Xb[rows, :].T)  # [E, 1024]
        m = {
            "xt": xt_np if var == "A" else xt_np_b,
            "xtloc": xtloc_np,
            "w2": W2,
        }
        (in_maps_a if var == "A" else in_maps_b).append(m)

    run_a, run_b = _get_runners()
    # dispatch both variants; execution overlaps on disjoint device sets
    arrs_a = run_a(in_maps_a)
    arrs_b = run_b(in_maps_b)
    out_a = np.asarray(arrs_a[0]).reshape(4, 1024, 512 * 16)
    out_b = np.asarray(arrs_b[0]).reshape(4, 1024, 512 * 12)

    # Unshard: copy each block's valid columns, apply the triangular mask on
    # the diagonal 128x128 sub-block, fill the rest with -inf.
    out_full = np.full((N, N), NEG_INF, dtype=np.float32)
    for c in range(NCORES):
        blocks, var = blocks_for_core(c)
        dev = out_a[c] if var == "A" else out_b[c - 4]
        for pos, b in enumerate(blocks):
            w = (b + 1) * P
            blk = out_full[b * P:(b + 1) * P]
            blk[:, :w] = dev[pos * P:(pos + 1) * P, :w]
            blk[:, b * P:(b + 1) * P][_TRI] = NEG_INF

    # Rank-1 bias terms (zero for this problem's inputs; exact in general).
    if np.any(bq) or np.any(bk):
        v2 = (X @ (bq @ Wk)) * scale  # per-column term
        s = ((X @ Wq.T + bq) * scale) @ bk  # per-row term
        out_full += s[:, None]
        out_full += v2[None, :]

    return out_full


# revision 11
# speedup vs baseline: 1.3250x; 1.0774x over previous
"""Trainium2 Bass kernel for nn_AnaphoricityScorer (masked attention logits).

Computes attn = causal_mask(q @ k.T) where
    q = (X @ Wq.T + bq) / sqrt(E),  k = X @ Wk.T + bk,  X = all_mentions
and the mask sets entries strictly above the diagonal to -inf.

Strategy (8 NeuronCores, 2 NEFF variants, per-core data):
  * Algebra: q @ k.T = X @ W2 @ X.T + s[r] + v2[n], with
        W2 = Wq.T @ Wk / sqrt(E)          (fused on host, tiny)
        s = q @ bk, v2 = (bq @ Wk @ X.T)/sqrt(E)  (rank-1 bias terms; zero for
        this problem's inputs, added on host when nonzero).
    The device runs ONE projection t = X_loc @ W2 plus one big t @ X.T.
  * Causal triangle: ~44% of the N x N output is -inf and is never computed.
    Row-blocks of 128 rows are assigned to cores so that each core's 8 blocks
    have column needs (in 512-col units) matching a fixed per-variant multiset:
        variant A (cores 0-3): {16,15,14,13,4,3,2,1}
        variant B (cores 4-7): {12,11,10,9,8,7,6,5}
    Both sum to 68 [128x512] output tiles -> identical wall time, and
    sum_b ceil((b+1)/4) = 544 matmuls/core is the uniform-graph minimum at
    512-column granularity.  The inner loop streams 512-column chunks of X.T
    in natural order; position p consumes chunks 0..S[p]-1.
  * Masking and -inf fill happen on the host during unsharding (the device
    computes only finite logits); compute is bf16 with fp32 PSUM accumulation.
  * A few dummy matmuls at kernel start warm the PE HAM clock-gate to 2.4GHz
    while the first input DMAs are still in flight.
"""

import sys

if "/opt/trn_rl_repo" not in sys.path:
    sys.path.insert(0, "/opt/trn_rl_repo")

import ml_dtypes
import numpy as np

BF16 = ml_dtypes.bfloat16
N, E, P, NCORES = 8192, 1024, 128, 8
NBLK = 8  # row-blocks of 128 rows per core
NEG_INF = np.float32(-np.inf)

# variant -> per-position column need in 512-col units (descending)
SLOTS = {
    "A": [16, 15, 14, 13, 4, 3, 2, 1],
    "B": [12, 11, 10, 9, 8, 7, 6, 5],
}


def blocks_for_core(c):
    """Row-blocks (of 128 rows) owned by core c, in position order."""
    var = "A" if c < 4 else "B"
    cc = c if c < 4 else c - 4
    return [4 * (k - 1) + cc for k in SLOTS[var]], var


_CACHE: dict = {}


def _build_nc(var):
    import concourse.mybir as mybir
    import concourse.tile as tile
    from concourse import bacc

    f32 = mybir.dt.float32
    bf = mybir.dt.bfloat16
    slots = SLOTS[var]
    nchunks = max(slots)  # 16 for A, 12 for B

    nc = bacc.Bacc("TRN2", target_bir_lowering=False, debug=False,
                   num_devices=4)
    xt = nc.dram_tensor("xt", [nchunks, E, 512], bf, kind="ExternalInput")
    xtloc = nc.dram_tensor("xtloc", [E, 1024], bf, kind="ExternalInput")
    w2 = nc.dram_tensor("w2", [E, 1024], bf, kind="ExternalInput")
    out = nc.dram_tensor("out", [1024, 512 * nchunks], f32,
                         kind="ExternalOutput")
    outap = out.ap()

    with tile.TileContext(nc) as tc:
        with (
            tc.tile_pool(name="const", bufs=1) as constp,
            tc.tile_pool(name="xtp", bufs=4) as xtp,
            tc.tile_pool(name="obp", bufs=6) as obp,
            tc.tile_pool(name="psp", bufs=8, space="PSUM") as psp,
        ):
            # PE warm-up: dummy matmuls on uninitialized SBUF flip the HAM
            # clock gate to 8/8 while the first input DMAs land.  Results go
            # to a scratch PSUM tile that is never read.
            wsrc = constp.tile([P, 512], bf)
            nc.vector.memset(wsrc[:], 0.0)
            wps0 = psp.tile([P, 512], f32, tag="ps")
            wps1 = psp.tile([P, 512], f32, tag="ps")
            for k in range(4):
                # alternate banks so fill overlaps drain (~216ns spacing)
                nc.tensor.matmul(wps0[:], wsrc[:, :P], wsrc[:],
                                 start=(k == 0), stop=(k == 3))
                nc.tensor.matmul(wps1[:], wsrc[:, :P], wsrc[:],
                                 start=(k == 0), stop=(k == 3))

            w2_sb = constp.tile([P, 8, 1024], bf)
            xl_sb = constp.tile([P, 8, 1024], bf)
            w2_ap = w2.ap().rearrange("(eo p) n -> p eo n", p=P)
            xl_ap = xtloc.ap().rearrange("(eo p) n -> p eo n", p=P)
            # per-e-chunk DMAs (scalar queue) so the first matmul can start
            # after ~512KB instead of 4MB
            for e in range(8):
                nc.sync.dma_start(w2_sb[:, e:e + 1, :], w2_ap[:, e:e + 1, :])
                nc.sync.dma_start(xl_sb[:, e:e + 1, :], xl_ap[:, e:e + 1, :])
            tt_sb = constp.tile([P, 8, 1024], bf)

            # Phase 1: tT = (X_loc @ W2).T, e-outer over 8 PSUM banks so the
            # PE starts on chunk 0 while later chunks are still in flight.
            for rh in range(2):
                pss = [psp.tile([P, 512], f32, name=f"ttps_{rh}_{k}", tag="ps")
                       for k in range(8)]
                for e in range(8):
                    for ic in range(8):
                        nc.tensor.matmul(
                            pss[ic][:],
                            w2_sb[:, e, ic * P:(ic + 1) * P],
                            xl_sb[:, e, rh * 512:(rh + 1) * 512],
                            start=(e == 0),
                            stop=(e == 7),
                        )
                for ic in range(8):
                    dst = tt_sb[:, ic, rh * 512:(rh + 1) * 512]
                    if ic % 2 == 0:
                        nc.vector.tensor_copy(dst, pss[ic][:])
                    else:
                        nc.scalar.copy(dst, pss[ic][:])

            # Phase 2: stream 512-col chunks of X.T; chunk t feeds every
            # position p with slots[p] > t.
            for t in range(nchunks):
                xt_sb = xtp.tile([P, 8, 512], bf)
                nc.sync.dma_start(
                    xt_sb[:], xt.ap()[t].rearrange("(eo p) n -> p eo n", p=P)
                )
                active = [pos for pos in range(8) if slots[pos] > t]
                # process positions in pairs so consecutive matmuls hit
                # alternating PSUM banks (same-bank accumulation serializes
                # fill against drain: 259ns/MM instead of 216ns/MM)
                for g in range(0, len(active), 4):
                    grp = active[g:g + 4]
                    pss = [psp.tile([P, 512], f32, name=f"aps_{t}_{g}_{k}",
                                    tag="ps") for k in range(len(grp))]
                    for ic in range(8):
                        for ps, pos in zip(pss, grp):
                            nc.tensor.matmul(
                                ps[:],
                                tt_sb[:, ic, pos * P:(pos + 1) * P],
                                xt_sb[:, ic, :],
                                start=(ic == 0),
                                stop=(ic == 7),
                            )
                    for ps, pos in zip(pss, grp):
                        ob = obp.tile([P, 512], f32)
                        if pos % 2 == 0:
                            nc.vector.tensor_copy(ob[:], ps[:])
                        else:
                            nc.scalar.copy(ob[:], ps[:])
                        nc.scalar.dma_start(
                            outap[pos * P:(pos + 1) * P,
                                  t * 512:(t + 1) * 512],
                            ob[:],
                        )
    nc.compile()
    return nc


def _make_exec(nc, devices):
    """Build a cached jitted executable for one variant (4 devices)."""
    import jax
    from jax.sharding import Mesh, PartitionSpec
    try:
        from jax.experimental.shard_map import shard_map
    except ImportError:
        from jax.shard_map import shard_map
    import concourse.mybir as mybir
    from concourse import bass2jax

    bass2jax.install_neuronx_cc_hook()

    partition_name = (nc.partition_id_tensor.name
                      if nc.partition_id_tensor else None)
    in_names, out_names, out_avals, zero_shapes = [], [], [], []
    for alloc in nc.m.functions[0].allocations:
        if not isinstance(alloc, mybir.MemoryLocationSet):
            continue
        name = alloc.memorylocations[0].name
        if alloc.kind == "ExternalInput":
            if name != partition_name:
                in_names.append(name)
        elif alloc.kind == "ExternalOutput":
            out_names.append(name)
            shape = tuple(alloc.tensor_shape)
            dtype = mybir.dt.np(alloc.dtype)
            out_avals.append(jax.core.ShapedArray(shape, dtype))
            zero_shapes.append((shape, dtype))
    n_params = len(in_names)
    all_names = in_names + out_names
    if partition_name is not None:
        all_names = all_names + [partition_name]
    donate = tuple(range(n_params, n_params + len(out_names)))

    def _body(*args):
        operands = list(args)
        if partition_name is not None:
            operands.append(bass2jax.partition_id_tensor())
        outs = bass2jax._bass_exec_p.bind(
            *operands,
            out_avals=tuple(out_avals),
            in_names=tuple(all_names),
            out_names=tuple(out_names),
            lowering_input_output_aliases=(),
            sim_require_finite=True,
            sim_require_nnan=True,
            nc=nc,
        )
        return tuple(outs)

    n_cores = len(devices)
    mesh = Mesh(np.asarray(devices), ("core",))
    specs = (PartitionSpec("core"),) * (n_params + len(out_names))
    jitted = jax.jit(
        shard_map(_body, mesh=mesh, in_specs=specs,
                  out_specs=(PartitionSpec("core"),) * len(out_names),
                  check_rep=False),
        donate_argnums=donate, keep_unused=True,
    )

    def run(in_maps):
        concat_in = [
            np.concatenate([np.asarray(m[k]) for m in in_maps], axis=0)
            for k in in_names
        ]
        zeros = [np.zeros((n_cores * s[0], *s[1:]), dt)
                 for (s, dt) in zero_shapes]
        out_arrs = jitted(*concat_in, *zeros)
        return out_arrs

    return run


def _get_runners():
    if "runners" not in _CACHE:
        import jax

        devs = jax.devices()
        nc_a = _build_nc("A")
        nc_b = _build_nc("B")
        _CACHE["runners"] = (
            _make_exec(nc_a, devs[0:4]),
            _make_exec(nc_b, devs[4:8]),
        )
    return _CACHE["runners"]


_TRI = np.triu(np.ones((P, P), dtype=bool), k=1)


def kernel(all_mentions, Wq, bq, Wk, bk):
    X = np.asarray(all_mentions, dtype=np.float32)
    Wq = np.asarray(Wq, dtype=np.float32)
    Wk = np.asarray(Wk, dtype=np.float32)
    bq = np.asarray(bq, dtype=np.float32)
    bk = np.asarray(bk, dtype=np.float32)

    scale = np.float32(1.0 / np.sqrt(E))
    W2 = ((Wq.T @ Wk) * scale).astype(BF16)  # [E, E] fused projection

    Xb = X.astype(BF16)
    XbT = np.ascontiguousarray(Xb.T)  # [E, N]
    # xt[t, e, m] = X[512 t + m, e]; variant B only consumes chunks 0..11
    xt_np = np.ascontiguousarray(XbT.reshape(E, 16, 512).swapaxes(0, 1))
    xt_np_b = np.ascontiguousarray(xt_np[:12])

    in_maps_a, in_maps_b = [], []
    for c in range(NCORES):
        blocks, var = blocks_for_core(c)
        rows = np.concatenate(
            [np.arange(b * P, (b + 1) * P) for b in blocks])
        xtloc_np = np.ascontiguousarray(Xb[rows, :].T)  # [E, 1024]
        m = {
            "xt": xt_np if var == "A" else xt_np_b,
            "xtloc": xtloc_np,
            "w2": W2,
        }
        (in_maps_a if var == "A" else in_maps_b).append(m)

    run_a, run_b = _get_runners()
    # dispatch both variants; execution overlaps on disjoint device sets
    arrs_a = run_a(in_maps_a)
    arrs_b = run_b(in_maps_b)
    out_a = np.asarray(arrs_a[0]).reshape(4, 1024, 512 * 16)
    out_b = np.asarray(arrs_b[0]).reshape(4, 1024, 512 * 12)

    # Unshard: copy each block's valid columns, apply the triangular mask on
    # the diagonal 128x128 sub-block, fill the rest with -inf.
    out_full = np.full((N, N), NEG_INF, dtype=np.float32)
    for c in range(NCORES):
        blocks, var = blocks_for_core(c)
        dev = out_a[c] if var == "A" else out_b[c - 4]
        for pos, b in enumerate(blocks):
            w = (b + 1) * P
            blk = out_full[b * P:(b + 1) * P]
            blk[:, :w] = dev[pos * P:(pos + 1) * P, :w]
            blk[:, b * P:(b + 1) * P][_TRI] = NEG_INF

    # Rank-1 bias terms (zero for this problem's inputs; exact in general).
    if np.any(bq) or np.any(bk):
        v2 = (X @ (bq @ Wk)) * scale  # per-column term
        s = ((X @ Wq.T + bq) * scale) @ bk  # per-row term
        out_full += s[:, None]
        out_full += v2[None, :]

    return out_full
